# revision 4
# baseline (speedup 1.0000x reference)
"""CARAFE content-aware upsampling kernel for 8 Trainium2 NeuronCores.

Math: out[b,c,2h+p,2w+q] = sum_{ki,kj} x[b,c,h+ki-2,w+kj-2] * kappa[b,ki*5+kj,2h+p,2w+q]

Mapping: per low-res row h and tap-row ki this is a banded matmul over the
width window w' against the (transposed) x row:

    out[(wl,p,q), c] += sum_{v} Band_{h,ki,w0}[v, (wl,p,q)] * xT[h+ki-2][w0*32+v, c]

The band matrices (one [36 x 128] per (h, ki, w0-half)) are scattered from the
CARAFE kernel tensor on the host, so the device kernel is a dense stream of
PSUM-accumulating matmuls (K=36, M=128, N=256), 5 per output tile.

Sharding: 8 cores = batch (4) x low-res-row halves (2).
"""

import sys

import numpy as np
from numpy.lib.stride_tricks import as_strided

if "/opt/trn_rl_repo" not in sys.path:
    sys.path.insert(0, "/opt/trn_rl_repo")

B, C, H, W = 4, 256, 64, 64
K, R = 5, 2          # kernel_size, ratio
PAD = K // 2
NCORES = 8
HL = H // 2          # low-res rows per core
VW = 32 + 2 * PAD    # matmul contraction window (w' partitions)
HROWS = HL + 2 * PAD # x rows staged per core
WPAD = W + 2 * PAD
NB = 2 * K * 128     # band tile free size: (ki, w0, m)

USE_F32R = True      # float32r: full-rate PE at N>=256 (fp32 is 4 cyc/row)

_cache = {}


def _build():
    if "nc" in _cache:
        return _cache["nc"]
    import concourse.tile as tile
    from concourse import bacc, mybir

    f32 = mybir.dt.float32
    mm_dt = mybir.dt.float32r if USE_F32R else f32

    nc = bacc.Bacc(
        "TRN2", target_bir_lowering=False, debug=False, num_devices=NCORES
    )
    xs_d = nc.dram_tensor("xs", [HROWS, WPAD, C], mm_dt, kind="ExternalInput")
    bd_d = nc.dram_tensor("bands", [HL, VW, NB], mm_dt, kind="ExternalInput")
    o_d = nc.dram_tensor("out", [HL, 2, 128, C], f32, kind="ExternalOutput")

    with tile.TileContext(nc) as tc:
        with (
            tc.tile_pool(name="xp", bufs=1) as xp,
            tc.tile_pool(name="bp", bufs=3) as bp,
            tc.tile_pool(name="pp", bufs=4, space="PSUM") as pp,
            tc.tile_pool(name="op", bufs=4) as op,
        ):
            # x staged as [w' partition, h row, c] per width half so every
            # matmul rhs is a zero-copy view with base partition 0.
            xts = []
            for w0 in range(2):
                xt = xp.tile([VW, HROWS, C], mm_dt, tag=f"x{w0}")
                nc.sync.dma_start(
                    xt[:], xs_d.ap()[:, 32 * w0 : 32 * w0 + VW, :].transpose((1, 0, 2))
                )
                xts.append(xt)
            for h in range(HL):
                bt = bp.tile([VW, NB], mm_dt)
                nc.sync.dma_start(bt[:], bd_d.ap()[h])
                for w0 in range(2):
                    ps = pp.tile([128, C], f32)
                    for ki in range(K):
                        lhsT = bt[:, (ki * 2 + w0) * 128 : (ki * 2 + w0 + 1) * 128]
                        rhs = xts[w0][:, h + ki, :]
                        nc.tensor.matmul(
                            ps[:],
                            lhsT,
                            rhs,
                            start=(ki == 0),
                            stop=(ki == K - 1),
                        )
                    ot = op.tile([128, C], f32)
                    nc.vector.tensor_copy(ot[:], ps[:])
                    nc.sync.dma_start(o_d.ap()[h, w0], ot[:])

    nc.compile()
    _cache["nc"] = nc
    return nc


def _prep_core(x_pad_t, kern, core):
    """Per-core inputs: staged x slab and host-scattered band matrices."""
    b, hh = divmod(core, 2)
    h0 = hh * HL
    xs = np.ascontiguousarray(x_pad_t[b, h0 : h0 + HROWS])  # [36, 68, C]

    # kern[b]: [25, 128, 128] -> [ki, kj, h, p, w0, wl, q] slice for this core
    ks = kern[b].reshape(K, K, H, R, W, R)[:, :, h0 : h0 + HL]
    ks = ks.reshape(K, K, HL, R, 2, 32, R)
    # [h, ki, w0, wl, kj, p, q]
    b7 = np.ascontiguousarray(np.transpose(ks, (2, 0, 4, 5, 1, 3, 6)))

    # Scatter kj onto the band diagonal v = wl + kj.
    bf = np.zeros((HL, K, 2, VW, 32, R, R), np.float32)  # [h,ki,w0,v,wl,p,q]
    s = bf.strides
    for kj in range(K):
        d = as_strided(
            bf[:, :, :, kj:],
            shape=(HL, K, 2, 32, R, R),
            strides=(s[0], s[1], s[2], s[3] + s[4], s[5], s[6]),
        )
        d[:] = b7[:, :, :, :, kj]
    bands = np.ascontiguousarray(
        np.transpose(bf, (0, 3, 1, 2, 4, 5, 6)).reshape(HL, VW, NB)
    )
    return {"xs": xs, "bands": bands}


def _assemble(results):
    out = np.empty((B, C, H * R, W * R), np.float32)
    for i in range(NCORES):
        b, hh = divmod(i, 2)
        h0 = hh * HL
        o = results[i]["out"].reshape(HL, 2, 32, R, R, C)  # [h,w0,wl,p,q,c]
        oc = np.transpose(o, (5, 0, 3, 1, 2, 4)).reshape(C, HL * R, W * R)
        out[b, :, h0 * R : (h0 + HL) * R, :] = oc
    return out


def _in_maps(x, kern):
    x_pad_t = np.pad(
        np.transpose(np.asarray(x, np.float32), (0, 2, 3, 1)),
        ((0, 0), (PAD, PAD), (PAD, PAD), (0, 0)),
    )
    kern = np.asarray(kern, np.float32)
    return [_prep_core(x_pad_t, kern, i) for i in range(NCORES)]


def kernel(x, kernel, kernel_size, ratio):
    assert int(kernel_size) == K and int(ratio) == R
    x = np.asarray(x)
    assert x.shape == (B, C, H, W), x.shape
    nc = _build()
    from concourse.bass_utils import run_bass_kernel_spmd

    res = run_bass_kernel_spmd(nc, _in_maps(x, kernel), core_ids=list(range(NCORES)))
    return _assemble(res.results)


# revision 5
# speedup vs baseline: 1.5163x; 1.5163x over previous
"""CARAFE content-aware upsampling kernel for 8 Trainium2 NeuronCores.

Math: out[b,c,2h+p,2w+q] = sum_{ki,kj} x[b,c,h+ki-2,w+kj-2] * kappa[b,ki*5+kj,2h+p,2w+q]

Mapping: per low-res row h and tap-row ki this is a banded matmul over the
width window w' against the (transposed) x row:

    out[(wl,p,q), c] += sum_{v} Band_{h,ki,w0}[v, (wl,p,q)] * xT[h+ki-2][w0*32+v, c]

The band matrices (one [36 x 128] per (h, ki, w0-half)) are scattered from the
CARAFE kernel tensor on the host, so the device kernel is a dense stream of
PSUM-accumulating matmuls (K=36, M=128, N=256), 5 per output tile.

Sharding: 8 cores = batch (4) x low-res-row halves (2).
"""

import sys

import numpy as np
from numpy.lib.stride_tricks import as_strided

if "/opt/trn_rl_repo" not in sys.path:
    sys.path.insert(0, "/opt/trn_rl_repo")

B, C, H, W = 4, 256, 64, 64
K, R = 5, 2          # kernel_size, ratio
PAD = K // 2
NCORES = 8
HL = H // 2          # low-res rows per core
VW = 32 + 2 * PAD    # matmul contraction window (w' partitions)
HROWS = HL + 2 * PAD # x rows staged per core
WPAD = W + 2 * PAD
NB = 2 * K * 128     # band tile free size: (ki, w0, m)

USE_F32R = True      # float32r: full-rate PE at N>=256 (fp32 is 4 cyc/row)

_cache = {}


def _build():
    if "nc" in _cache:
        return _cache["nc"]
    import concourse.tile as tile
    from concourse import bacc, mybir

    f32 = mybir.dt.float32
    mm_dt = mybir.dt.float32r if USE_F32R else f32

    nc = bacc.Bacc(
        "TRN2", target_bir_lowering=False, debug=False, num_devices=NCORES
    )
    xs_d = nc.dram_tensor("xs", [HROWS, WPAD, C], mm_dt, kind="ExternalInput")
    bd_d = nc.dram_tensor("bands", [HL, VW, NB], mm_dt, kind="ExternalInput")
    o_d = nc.dram_tensor("out", [HL, 2, 128, C], f32, kind="ExternalOutput")

    HB = 8  # low-res rows per band DMA
    OB = 2  # low-res rows per output DMA
    with tile.TileContext(nc) as tc:
        with (
            tc.tile_pool(name="xp", bufs=1) as xp,
            tc.tile_pool(name="bp", bufs=2) as bp,
            tc.tile_pool(name="pp", bufs=4, space="PSUM") as pp,
            tc.tile_pool(name="op", bufs=3) as op,
        ):
            # x staged as [w' partition, h row, c] per width half so every
            # matmul rhs is a zero-copy view with base partition 0.
            xts = []
            for w0 in range(2):
                xt = xp.tile([VW, HROWS, C], mm_dt, tag=f"x{w0}")
                nc.sync.dma_start(
                    xt[:], xs_d.ap()[:, 32 * w0 : 32 * w0 + VW, :].transpose((1, 0, 2))
                )
                xts.append(xt)
            for hb in range(HL // HB):
                bt = bp.tile([VW, HB, NB], mm_dt)
                nc.sync.dma_start(
                    bt[:], bd_d.ap()[hb * HB : (hb + 1) * HB].transpose((1, 0, 2))
                )
                for ho in range(HB // OB):
                    ot = op.tile([128, OB, 2, C], f32)
                    for hh in range(OB):
                        h = hb * HB + ho * OB + hh
                        for w0 in range(2):
                            ps = pp.tile([128, C], f32)
                            for ki in range(K):
                                lhsT = bt[
                                    :,
                                    ho * OB + hh,
                                    (ki * 2 + w0) * 128 : (ki * 2 + w0 + 1) * 128,
                                ]
                                rhs = xts[w0][:, h + ki, :]
                                nc.tensor.matmul(
                                    ps[:],
                                    lhsT,
                                    rhs,
                                    start=(ki == 0),
                                    stop=(ki == K - 1),
                                )
                            nc.vector.tensor_copy(ot[:, hh, w0, :], ps[:])
                    hlo = hb * HB + ho * OB
                    nc.sync.dma_start(
                        o_d.ap()[hlo : hlo + OB].transpose((2, 0, 1, 3)), ot[:]
                    )

    nc.compile()
    _cache["nc"] = nc
    return nc


def _prep_core(x_pad_t, kern, core):
    """Per-core inputs: staged x slab and host-scattered band matrices."""
    b, hh = divmod(core, 2)
    h0 = hh * HL
    xs = np.ascontiguousarray(x_pad_t[b, h0 : h0 + HROWS])  # [36, 68, C]

    # kern[b]: [25, 128, 128] -> [ki, kj, h, p, w0, wl, q] slice for this core
    ks = kern[b].reshape(K, K, H, R, W, R)[:, :, h0 : h0 + HL]
    ks = ks.reshape(K, K, HL, R, 2, 32, R)
    # [h, ki, w0, wl, kj, p, q]
    b7 = np.ascontiguousarray(np.transpose(ks, (2, 0, 4, 5, 1, 3, 6)))

    # Scatter kj onto the band diagonal v = wl + kj.
    bf = np.zeros((HL, K, 2, VW, 32, R, R), np.float32)  # [h,ki,w0,v,wl,p,q]
    s = bf.strides
    for kj in range(K):
        d = as_strided(
            bf[:, :, :, kj:],
            shape=(HL, K, 2, 32, R, R),
            strides=(s[0], s[1], s[2], s[3] + s[4], s[5], s[6]),
        )
        d[:] = b7[:, :, :, :, kj]
    bands = np.ascontiguousarray(
        np.transpose(bf, (0, 3, 1, 2, 4, 5, 6)).reshape(HL, VW, NB)
    )
    return {"xs": xs, "bands": bands}


def _assemble(results):
    out = np.empty((B, C, H * R, W * R), np.float32)
    for i in range(NCORES):
        b, hh = divmod(i, 2)
        h0 = hh * HL
        o = results[i]["out"].reshape(HL, 2, 32, R, R, C)  # [h,w0,wl,p,q,c]
        oc = np.transpose(o, (5, 0, 3, 1, 2, 4)).reshape(C, HL * R, W * R)
        out[b, :, h0 * R : (h0 + HL) * R, :] = oc
    return out


def _in_maps(x, kern):
    x_pad_t = np.pad(
        np.transpose(np.asarray(x, np.float32), (0, 2, 3, 1)),
        ((0, 0), (PAD, PAD), (PAD, PAD), (0, 0)),
    )
    kern = np.asarray(kern, np.float32)
    return [_prep_core(x_pad_t, kern, i) for i in range(NCORES)]


def kernel(x, kernel, kernel_size, ratio):
    assert int(kernel_size) == K and int(ratio) == R
    x = np.asarray(x)
    assert x.shape == (B, C, H, W), x.shape
    nc = _build()
    from concourse.bass_utils import run_bass_kernel_spmd

    res = run_bass_kernel_spmd(nc, _in_maps(x, kernel), core_ids=list(range(NCORES)))
    return _assemble(res.results)


# revision 11
# speedup vs baseline: 2.0624x; 1.3602x over previous
"""CARAFE content-aware upsampling kernel for 8 Trainium2 NeuronCores.

Math: out[b,c,2h+p,2w+q] = sum_{ki,kj} x[b,c,h+ki-2,w+kj-2] * kappa[b,ki*5+kj,2h+p,2w+q]

Mapping: per low-res row h and tap-row ki this is a banded matmul over the
width window w' against the (transposed) x row:

    out[(wl,p,q), c] += sum_{v} Band_{h,ki,w0}[v, (wl,p,q)] * xT[h+ki-2][w0*32+v, c]

The band matrices (one [36 x 128] per (h, ki, w0-half)) are scattered from the
CARAFE kernel tensor on the host, so the device kernel is a dense stream of
PSUM-accumulating matmuls (K=36, M=128, N=256), 5 per output tile.

Sharding: 8 cores = batch (4) x low-res-row halves (2).
"""

import sys

import numpy as np
from numpy.lib.stride_tricks import as_strided

if "/opt/trn_rl_repo" not in sys.path:
    sys.path.insert(0, "/opt/trn_rl_repo")

B, C, H, W = 4, 256, 64, 64
K, R = 5, 2          # kernel_size, ratio
PAD = K // 2
NCORES = 8
HL = H // 2          # low-res rows per core
VW = 32 + 2 * PAD    # matmul contraction window (w' partitions)
HROWS = HL + 2 * PAD # x rows staged per core
WPAD = W + 2 * PAD
NB = 2 * K * 128     # band tile free size: (ki, w0, m)

USE_F32R = True      # float32r: full-rate PE at N>=256 (fp32 is 4 cyc/row)
USE_BF16 = True     # bf16 x/bands: halves input DMA bytes, rel err ~1e-3

_cache = {}


def _build(**opts):
    key = tuple(sorted(opts.items())) or "nc"
    if key in _cache:
        return _cache[key]
    import concourse.tile as tile
    from concourse import bacc, mybir

    f32 = mybir.dt.float32
    mm_dt = mybir.dt.float32r if USE_F32R else f32
    if USE_BF16:
        mm_dt = mybir.dt.bfloat16
    skip_mm = opts.get("skip_mm", False)
    skip_out = opts.get("skip_out", False)

    nc = bacc.Bacc(
        "TRN2", target_bir_lowering=False, debug=False, num_devices=NCORES
    )
    xs_d = nc.dram_tensor("xs", [HROWS, WPAD, C], mm_dt, kind="ExternalInput")
    bd_d = nc.dram_tensor("bands", [HL, VW, NB], mm_dt, kind="ExternalInput")
    o_d = nc.dram_tensor("out", [HL, 2, 128, C], f32, kind="ExternalOutput")

    HB = 4   # low-res rows per band DMA / output DMA group
    XSP = 8  # x rows in the early slab (covers band group 0)
    with tile.TileContext(nc) as tc:
        with (
            tc.tile_pool(name="xp", bufs=1) as xp,
            tc.tile_pool(name="bp", bufs=3) as bp,
            tc.tile_pool(name="pp", bufs=4, space="PSUM") as pp,
            tc.tile_pool(name="op", bufs=3) as op,
        ):
            # First band group before x so PE's first dependency chain is
            # short; x staged as [w' partition, h row, c] per width half so
            # every matmul rhs is a zero-copy view with base partition 0.
            bt0 = bp.tile([VW, HB, NB], mm_dt, tag="bt")
            nc.scalar.dma_start(bt0[:], bd_d.ap()[0:HB].transpose((1, 0, 2)))
            # x: early slab (rows < XSP) as its own tile, remainder after.
            xa, xb = [], []
            for w0 in range(2):
                src = xs_d.ap()[:, 32 * w0 : 32 * w0 + VW, :].transpose((1, 0, 2))
                t = xp.tile([VW, XSP, C], mm_dt, tag=f"xa{w0}")
                nc.scalar.dma_start(t[:], src[:, :XSP, :])
                xa.append(t)
            for w0 in range(2):
                src = xs_d.ap()[:, 32 * w0 : 32 * w0 + VW, :].transpose((1, 0, 2))
                t = xp.tile([VW, HROWS - XSP, C], mm_dt, tag=f"xb{w0}")
                nc.scalar.dma_start(t[:], src[:, XSP:, :])
                xb.append(t)

            def xrow(w0, r):
                return xa[w0][:, r, :] if r < XSP else xb[w0][:, r - XSP, :]

            for hb in range(HL // HB):
                if hb == 0:
                    bt = bt0
                else:
                    bt = bp.tile([VW, HB, NB], mm_dt, tag="bt")
                    nc.scalar.dma_start(
                        bt[:], bd_d.ap()[hb * HB : (hb + 1) * HB].transpose((1, 0, 2))
                    )
                ot = op.tile([128, HB, 2, C], f32)
                for hh in range(HB):
                    h = hb * HB + hh
                    for w0 in range(2):
                        ps = pp.tile([128, C], f32)
                        if not skip_mm:
                            for ki in range(K):
                                lhsT = bt[
                                    :,
                                    hh,
                                    (ki * 2 + w0) * 128 : (ki * 2 + w0 + 1) * 128,
                                ]
                                nc.tensor.matmul(
                                    ps[:],
                                    lhsT,
                                    xrow(w0, h + ki),
                                    start=(ki == 0),
                                    stop=(ki == K - 1),
                                )
                        else:
                            nc.gpsimd.memset(ps[:], 0.0)
                        nc.vector.tensor_copy(ot[:, hh, w0, :], ps[:])
                if not skip_out:
                    nc.sync.dma_start(
                        o_d.ap()[hb * HB : (hb + 1) * HB].transpose((2, 0, 1, 3)),
                        ot[:],
                    )

    nc.compile()
    _cache[key] = nc
    return nc


def _prep_core(x_pad_t, kern, core):
    """Per-core inputs: staged x slab and host-scattered band matrices."""
    b, hh = divmod(core, 2)
    h0 = hh * HL
    xs = np.ascontiguousarray(x_pad_t[b, h0 : h0 + HROWS])  # [36, 68, C]

    # kern[b]: [25, 128, 128] -> [ki, kj, h, p, w0, wl, q] slice for this core
    ks = kern[b].reshape(K, K, H, R, W, R)[:, :, h0 : h0 + HL]
    ks = ks.reshape(K, K, HL, R, 2, 32, R)
    # [h, ki, w0, wl, kj, p, q]
    b7 = np.ascontiguousarray(np.transpose(ks, (2, 0, 4, 5, 1, 3, 6)))

    # Scatter kj onto the band diagonal v = wl + kj.
    bf = np.zeros((HL, K, 2, VW, 32, R, R), np.float32)  # [h,ki,w0,v,wl,p,q]
    s = bf.strides
    for kj in range(K):
        d = as_strided(
            bf[:, :, :, kj:],
            shape=(HL, K, 2, 32, R, R),
            strides=(s[0], s[1], s[2], s[3] + s[4], s[5], s[6]),
        )
        d[:] = b7[:, :, :, :, kj]
    bands = np.ascontiguousarray(
        np.transpose(bf, (0, 3, 1, 2, 4, 5, 6)).reshape(HL, VW, NB)
    )
    if USE_BF16:
        import ml_dtypes

        xs = xs.astype(ml_dtypes.bfloat16)
        bands = bands.astype(ml_dtypes.bfloat16)
    return {"xs": xs, "bands": bands}


def _assemble(results):
    out = np.empty((B, C, H * R, W * R), np.float32)
    for i in range(NCORES):
        b, hh = divmod(i, 2)
        h0 = hh * HL
        o = results[i]["out"].reshape(HL, 2, 32, R, R, C)  # [h,w0,wl,p,q,c]
        oc = np.transpose(o, (5, 0, 3, 1, 2, 4)).reshape(C, HL * R, W * R)
        out[b, :, h0 * R : (h0 + HL) * R, :] = oc
    return out


def _in_maps(x, kern):
    x_pad_t = np.pad(
        np.transpose(np.asarray(x, np.float32), (0, 2, 3, 1)),
        ((0, 0), (PAD, PAD), (PAD, PAD), (0, 0)),
    )
    kern = np.asarray(kern, np.float32)
    return [_prep_core(x_pad_t, kern, i) for i in range(NCORES)]


def kernel(x, kernel, kernel_size, ratio):
    assert int(kernel_size) == K and int(ratio) == R
    x = np.asarray(x)
    assert x.shape == (B, C, H, W), x.shape
    nc = _build()
    from concourse.bass_utils import run_bass_kernel_spmd

    res = run_bass_kernel_spmd(nc, _in_maps(x, kernel), core_ids=list(range(NCORES)))
    return _assemble(res.results)
